# revision 1
# baseline (speedup 1.0000x reference)
"""Trainium2 Bass kernel for CoEncoderDynamicAttention (v2: bf16 datapath,
3-engine exp split, PSUM->DRAM output).

Model (reference):
  q = x @ wq   -> [B,S,NH,HD];  k = x @ wk -> [B,S,NKV,HD];  v = x @ wv
  scores = q k^T / sqrt(HD), masked, softmax over k
  out = (attn @ v) reshaped @ wo        (wo: [NH*HD, 1])

Sharding: 8 cores = (batch b in 0..1) x (kv-group g in 0..3).  Each kv
group owns 1 kv head and GQ=4 q heads.  Since wo has output dim 1, fold
wo into v on the host:  u_h = v_g @ wo_h, so per-core output is
  num_h[q] = sum_k m[k] u_h[k] e_h[k,q],  den_h[q] = sum_k m[k] e_h[k,q]
with e = exp(s/sqrt(HD)); the mask is folded multiplicatively into the
AV stationary operand (no -inf bias needed).  Host combines
out[b,q] = sum_{g,h} num_h/den_h.

Precision: num (and q, k, u themselves) are cancelling sums over
random-sign weights, so per-element operand noise passes straight to
the output -- every matmul operand must be ~16-bit.  All matmuls run
bf16 (1.0 PE cycles/row, same rate as fp32r but half the SBUF/DMA
traffic).

exp split: 9/16 of k-tiles on the ACT engine (native Exp, bf16 out),
4/16 on DVE and 3/16 on GPSIMD via the Schraudolph bit trick in bf16:
u16 = round(ps*A + B) written as uint16 IS the bf16 encoding of
~exp(s) (mean-centered bit-linear approx, ~2% rms sawtooth, diluted by
the accurate tiles and partially cancelling in num/den).

Other perf structure:
  * Scores matmuls pair the two heads of a q-head pair in disjoint PE
    row groups (KT duplicated to partitions 64-127) so both stream
    concurrently.
  * num/den accumulate in PSUM across k-tiles and are DMA'd straight
    to DRAM (no engine copy).
  * AV matmul for k-tile t is emitted after the scores matmuls of
    k-tile t+1 so the tensor queue never waits on the exp engines.
"""

import numpy as np
import ml_dtypes

import concourse.bass as bass
import concourse.mybir as mybir
import concourse.tile as tile
from concourse.bass_utils import run_bass_kernel_spmd

B, S, H = 2, 2048, 1024
NH, NKV, HD = 16, 4, 64
GQ = NH // NKV          # q heads per kv group
EW = GQ * HD            # per-core q projection width (256)
NCORES = 8
P = 128
HT = H // P             # h (contraction) tiles
F32 = mybir.dt.float32
BF16 = mybir.dt.bfloat16
U16 = mybir.dt.uint16
AF = mybir.ActivationFunctionType
OP = mybir.AluOpType
NPBF = ml_dtypes.bfloat16

# Schraudolph-to-bf16: for psum score ps (= 8*s_true),
# u16 = round(ps*SCH_A + SCH_B) is the bf16 bit pattern of ~exp(s_true).
# SCH_A = 128*log2(e)/8; SCH_B = 127*128 - 0.0576*128 (mean-centers the
# +4.07% bit-linear interpolation bias).
SCH_A = float(128.0 * np.log2(np.e) / 8.0)
SCH_B = float(16256.0 - 7.373)

# per-(kt % 16) exp engine: 10 ACT, 6 DVE (GPSIMD cannot read PSUM)
EXP_ASSIGN = ["act", "act", "dve", "act", "dve", "act", "dve", "act",
              "act", "dve", "act", "dve", "act", "act", "dve", "act"]


def _split_excess_waits(nc, limit=1):
    """This walrus build only accepts one sync-wait (and update) per
    instruction; hoist extras onto NoOps on the same engine."""
    for f in nc.m.functions:
        for bb in f.blocks:
            new = []
            for inst in bb.instructions:
                si = getattr(inst, "sync_info", None)
                waits = list(si.on_wait) if (si is not None and si.on_wait) else []
                k = 0
                while len(waits) > limit:
                    chunk, waits = waits[:limit], waits[limit:]
                    nop = mybir.InstNoOp(name=f"{inst.name}-ws{k}", ins=[], outs=[])
                    nop.engine = inst.engine
                    nop.sync_info = mybir.SyncInfo(on_wait=chunk, on_update=[])
                    nc.register_instruction(nop)
                    new.append(nop)
                    k += 1
                if k:
                    si.on_wait = waits
                new.append(inst)
                ups = list(si.on_update) if (si is not None and si.on_update) else []
                if len(ups) > limit and type(inst).__name__ not in (
                    "InstDMA", "InstDMACopy", "InstTensorLoad", "InstTensorSave",
                ):
                    si.on_update = ups[:limit]
                    for j, up in enumerate(ups[limit:]):
                        nop = mybir.InstNoOp(name=f"{inst.name}-us{j}", ins=[], outs=[])
                        nop.engine = inst.engine
                        nop.sync_info = mybir.SyncInfo(on_wait=[], on_update=[up])
                        nc.register_instruction(nop)
                        new.append(nop)
            bb.instructions[:] = new


def build_nc(s=S, repeat=1):
    st = s // P             # number of 128-wide k tiles
    qc_w = min(512, s)      # q chunk width
    nqc = s // qc_w

    nc = bass.Bass()
    xb = nc.dram_tensor("xb", [P, HT, s], BF16, kind="ExternalInput")
    wqb = nc.dram_tensor("wqb", [P, HT, EW], BF16, kind="ExternalInput")
    wkub = nc.dram_tensor("wkub", [P, HT, HD + GQ], BF16, kind="ExternalInput")
    mkf = nc.dram_tensor("mkf", [P, st], F32, kind="ExternalInput")
    uscr = nc.dram_tensor("uscr", [GQ, s], F32)
    out = nc.dram_tensor("out", [GQ, 2, s], F32, kind="ExternalOutput")

    with tile.TileContext(nc) as tc:
        with (
            tc.tile_pool(name="persist", bufs=1) as persist,
            tc.tile_pool(name="ep", bufs=3) as ep,
            tc.tile_pool(name="obp", bufs=2) as obp,
            tc.tile_pool(name="psum_s", bufs=3, space="PSUM") as psum_s,
            tc.tile_pool(name="psum_o", bufs=1, space="PSUM") as psum_o,
        ):
            xb_sb = persist.tile([P, HT, s], BF16)
            wqb_sb = persist.tile([P, HT, EW], BF16)
            wkub_sb = persist.tile([P, HT, HD + GQ], BF16)
            mkf_sb = persist.tile([P, st], F32)
            KT2 = persist.tile([P, s], BF16)
            QT = [persist.tile([P, s], BF16, tag=f"qt_{i}", name=f"qt_{i}")
                  for i in range(2)]
            MUB = persist.tile([P, st, 2 * GQ], BF16)
            UST = persist.tile([P, st, GQ], F32)   # u scatter staging
            USTF = persist.tile([P, s], F32)     # u psum->sbuf staging (rows 64-67)

            nc.sync.dma_start(out=wqb_sb[:], in_=wqb[:, :, :])
            nc.sync.dma_start(out=wkub_sb[:], in_=wkub[:, :, :])
            nc.sync.dma_start(out=mkf_sb[:], in_=mkf[:, :])
            for q in range(nqc):
                sl = slice(q * qc_w, (q + 1) * qc_w)
                nc.sync.dma_start(out=xb_sb[:, :, sl], in_=xb[:, :, sl])

            for _ in range(repeat):
                # ---- K+U projection ----
                for q in range(nqc):
                    sl = slice(q * qc_w, (q + 1) * qc_w)
                    ps = psum_s.tile([P, 2, qc_w], F32, tag="ps")
                    for t in range(HT):
                        nc.tensor.matmul(
                            ps[0:HD + GQ, 0, :], lhsT=wkub_sb[:, t, :],
                            rhs=xb_sb[:, t, sl],
                            start=(t == 0), stop=(t == HT - 1))
                    nc.vector.tensor_copy(KT2[0:HD, sl], ps[0:HD, 0, :])
                    # u rows (64-67) bounce through DRAM for the k-scatter
                    nc.vector.tensor_copy(USTF[HD:HD + GQ, sl],
                                          ps[HD:HD + GQ, 0, :])
                    nc.sync.dma_start(out=uscr[:, sl],
                                      in_=USTF[HD:HD + GQ, sl])
                # duplicate k^T to rows 64-127 (concurrent head pair matmuls)
                nc.sync.dma_start(out=KT2[HD:P, :], in_=KT2[0:HD, :])

                # MUB: u columns (k-scatter to partition-major, mask, cast)
                for j in range(GQ):
                    nc.sync.dma_start(
                        out=UST[:, :, j], in_=uscr[j, :].rearrange("(t p) -> p t", p=P))
                    nc.gpsimd.tensor_tensor(
                        out=MUB[:, :, 2 * j], in0=UST[:, :, j], in1=mkf_sb[:, :],
                        op=OP.mult)
                    nc.gpsimd.tensor_copy(MUB[:, :, 2 * j + 1], mkf_sb[:, :])

                # ---- Q projection ----
                for p2 in range(2):
                    for q in range(nqc):
                        sl = slice(q * qc_w, (q + 1) * qc_w)
                        psq = psum_s.tile([P, 2, qc_w], F32, tag="ps")
                        for t in range(HT):
                            nc.tensor.matmul(
                                psq[:, 0, :],
                                lhsT=wqb_sb[:, t, p2 * P:(p2 + 1) * P],
                                rhs=xb_sb[:, t, sl],
                                start=(t == 0), stop=(t == HT - 1))
                        nc.vector.tensor_copy(QT[p2][:, sl], psq[:, 0, :])

                # ---- attention ----
                for hp in range(2):
                    c0, c1 = 2 * (2 * hp), 2 * (2 * hp + 1)
                    for q in range(nqc):
                        qsl = slice(q * qc_w, (q + 1) * qc_w)
                        po = psum_o.tile([2, 2, qc_w], F32, tag="po")
                        prev = None
                        for kt in range(st):
                            ksl = slice(kt * P, (kt + 1) * P)
                            ps = psum_s.tile([P, 2, qc_w], F32, tag="ps")
                            nc.tensor.matmul(
                                ps[:, 0, :], lhsT=KT2[0:HD, ksl],
                                rhs=QT[hp][0:HD, qsl], start=True, stop=True,
                                tile_position=(0, 0))
                            nc.tensor.matmul(
                                ps[:, 1, :], lhsT=KT2[HD:P, ksl],
                                rhs=QT[hp][HD:P, qsl], start=True, stop=True,
                                tile_position=(HD, 0))
                            e = ep.tile([P, 2, qc_w], BF16, tag="e")
                            eng = EXP_ASSIGN[kt % 16]
                            if eng == "act":
                                nc.scalar.activation(
                                    e[:, :, :], ps[:, :, :], AF.Exp,
                                    scale=1.0 / 8.0)
                            elif eng == "dve":
                                nc.vector.tensor_scalar(
                                    out=e[:, :, :].bitcast(U16),
                                    in0=ps[:, :, :], scalar1=SCH_A,
                                    scalar2=SCH_B, op0=OP.mult, op1=OP.add)
                            else:
                                nc.gpsimd.tensor_scalar(
                                    out=e[:, :, :].bitcast(U16),
                                    in0=ps[:, :, :], scalar1=SCH_A,
                                    scalar2=SCH_B, op0=OP.mult, op1=OP.add)
                            if prev is not None:
                                pk, pe = prev
                                nc.tensor.matmul(
                                    po[:, 0, :], lhsT=MUB[:, pk, c0:c0 + 2],
                                    rhs=pe[:, 0, :], start=(pk == 0), stop=False)
                                nc.tensor.matmul(
                                    po[:, 1, :], lhsT=MUB[:, pk, c1:c1 + 2],
                                    rhs=pe[:, 1, :], start=(pk == 0), stop=False)
                            prev = (kt, e)
                        pk, pe = prev
                        nc.tensor.matmul(
                            po[:, 0, :], lhsT=MUB[:, pk, c0:c0 + 2],
                            rhs=pe[:, 0, :], start=(pk == 0), stop=True)
                        nc.tensor.matmul(
                            po[:, 1, :], lhsT=MUB[:, pk, c1:c1 + 2],
                            rhs=pe[:, 1, :], start=(pk == 0), stop=True)
                        ob = obp.tile([2, 2, qc_w], F32, tag="ob")
                        nc.vector.tensor_copy(ob[:, :, :], po[:, :, :])
                        nc.sync.dma_start(
                            out=out[2 * hp:2 * hp + 2, :, qsl].rearrange(
                                "h n q -> n h q"),
                            in_=ob[:, :, :])

    _split_excess_waits(nc)
    return nc


_NC_CACHE = {}


def _get_nc(s=S, repeat=1):
    key = (s, repeat)
    if key not in _NC_CACHE:
        _NC_CACHE[key] = build_nc(s, repeat)
    return _NC_CACHE[key]


def make_inputs(hidden_states, attention_mask, wq, wk, wv, wo, s=S):
    """Host-side shard prep: per-core input dicts (bf16, h-tiled)."""
    hidden_states = np.asarray(hidden_states, dtype=np.float32)
    attention_mask = np.asarray(attention_mask)
    wq = np.asarray(wq, dtype=np.float32)
    wk = np.asarray(wk, dtype=np.float32)
    wv = np.asarray(wv, dtype=np.float32)
    wo = np.asarray(wo, dtype=np.float32)
    st = s // P

    def h_pack(a):
        # [H, C] -> [128, HT, C]  with h = 128*t + j
        c = a.shape[1]
        return np.ascontiguousarray(a.reshape(HT, P, c).transpose(1, 0, 2))

    in_maps = []
    for core in range(NCORES):
        b, g = divmod(core, NKV)
        xT = np.ascontiguousarray(hidden_states[b, :s, :].T)      # [H, s]
        xb = h_pack(xT).astype(NPBF)
        wq_g = wq[:, g * EW:(g + 1) * EW]
        wqb = h_pack(wq_g).astype(NPBF)
        wk_g = wk[:, g * HD:(g + 1) * HD]
        wo_g = wo[g * EW:(g + 1) * EW, 0].reshape(GQ, HD).T        # [HD, GQ]
        wu_g = wv[:, g * HD:(g + 1) * HD] @ wo_g
        wkub = h_pack(np.concatenate([wk_g, wu_g], axis=1)).astype(NPBF)
        m = (attention_mask[b, :s] != 0).astype(np.float32)
        mkf = np.ascontiguousarray(m.reshape(st, P).T)             # [128, st]
        in_maps.append({"xb": xb, "wqb": wqb, "wkub": wkub, "mkf": mkf})
    return in_maps


def combine(results, s=S):
    """Host-side gather: out[b,q] = sum over group cores and heads num/den."""
    out = np.zeros((B, s, 1), dtype=np.float32)
    for core in range(NCORES):
        b = core // NKV
        nd = results[core]["out"]          # [GQ, 2, s]
        out[b, :, 0] += (nd[:, 0, :] / nd[:, 1, :]).sum(axis=0)
    return out


def kernel(hidden_states, attention_mask, wq, wk, wv, wo):
    nc = _get_nc()
    in_maps = make_inputs(hidden_states, attention_mask, wq, wk, wv, wo)
    res = run_bass_kernel_spmd(nc, in_maps, core_ids=list(range(NCORES)))
    return combine(res.results)



# revision 3
# speedup vs baseline: 1.2763x; 1.2763x over previous
"""Trainium2 Bass kernel for CoEncoderDynamicAttention (v3: col-tiled AV,
1-bank PSUM accumulator, rebalanced exp split).

Model (reference):
  q = x @ wq   -> [B,S,NH,HD];  k = x @ wk -> [B,S,NKV,HD];  v = x @ wv
  scores = q k^T / sqrt(HD), masked, softmax over k
  out = (attn @ v) reshaped @ wo        (wo: [NH*HD, 1])

Sharding: 8 cores = (batch b in 0..1) x (kv-group g in 0..3).  Each kv
group owns 1 kv head and GQ=4 q heads.  Since wo has output dim 1, fold
wo into v on the host:  u_h = v_g @ wo_h, so per-core output is
  num_h[q] = sum_k m[k] u_h[k] e_h[k,q],  den_h[q] = sum_k m[k] e_h[k,q]
with e = exp(s/sqrt(HD)); the mask is folded multiplicatively into the
AV stationary operand.  Host combines out[b,q] = sum_{g,h} num_h/den_h.

Perf structure vs v2:
  * Scores matmuls pair heads in disjoint PE row groups (KT duplicated to
    partitions 64-127), both pairs (heads 0,1 then 2,3) per k-tile.
  * AV matmuls (M=2: num/den) are 4x COLUMN-TILED: head h ->
    tile_position (0, 32h), output partitions 32h..32h+1 of ONE 1-bank
    PSUM accumulator.  All four stream concurrently on disjoint PE
    column groups, so AV costs ~512 cycles/k-tile instead of ~2048.
  * exp split ACT/DVE tuned so both engines finish together; projection
    casts go to ACT (idle during the projection lead-in anyway).
  * AV for k-tile t is emitted after the scores matmuls of k-tile t+1 so
    the tensor queue never waits on the exp engines.

Precision: all matmuls bf16 (operand noise passes straight to the output
through the cancelling num sums).  DVE share of exp uses the Schraudolph
bit trick in bf16: u16 = round(ps*A + B) written as uint16 IS the bf16
encoding of ~exp(s) (~2% rms sawtooth, diluted across tiles).
"""

import numpy as np
import ml_dtypes

import concourse.bass as bass
import concourse.mybir as mybir
import concourse.tile as tile
from concourse.bass_utils import run_bass_kernel_spmd

B, S, H = 2, 2048, 1024
NH, NKV, HD = 16, 4, 64
GQ = NH // NKV          # q heads per kv group
EW = GQ * HD            # per-core q projection width (256)
NCORES = 8
P = 128
HT = H // P             # h (contraction) tiles
F32 = mybir.dt.float32
BF16 = mybir.dt.bfloat16
U16 = mybir.dt.uint16
AF = mybir.ActivationFunctionType
OP = mybir.AluOpType
NPBF = ml_dtypes.bfloat16

# Schraudolph-to-bf16: for psum score ps (= 8*s_true),
# u16 = round(ps*SCH_A + SCH_B) is the bf16 bit pattern of ~exp(s_true).
SCH_A = float(128.0 * np.log2(np.e) / 8.0)
SCH_B = float(16256.0 - 7.373)

# per-exp-tile engine pick, indexed by (2*kt + pair) % 16: 9 ACT, 7 DVE
EXP_ASSIGN = ["act", "dve", "act", "act", "dve", "act", "dve", "act",
              "dve", "act", "dve", "act", "act", "dve", "act", "act"]


def _split_excess_waits(nc, limit=1):
    """This walrus build only accepts one sync-wait (and update) per
    instruction; hoist extras onto NoOps on the same engine."""
    for f in nc.m.functions:
        for bb in f.blocks:
            new = []
            for inst in bb.instructions:
                si = getattr(inst, "sync_info", None)
                waits = list(si.on_wait) if (si is not None and si.on_wait) else []
                k = 0
                while len(waits) > limit:
                    chunk, waits = waits[:limit], waits[limit:]
                    nop = mybir.InstNoOp(name=f"{inst.name}-ws{k}", ins=[], outs=[])
                    nop.engine = inst.engine
                    nop.sync_info = mybir.SyncInfo(on_wait=chunk, on_update=[])
                    nc.register_instruction(nop)
                    new.append(nop)
                    k += 1
                if k:
                    si.on_wait = waits
                new.append(inst)
                ups = list(si.on_update) if (si is not None and si.on_update) else []
                if len(ups) > limit and type(inst).__name__ not in (
                    "InstDMA", "InstDMACopy", "InstTensorLoad", "InstTensorSave",
                ):
                    si.on_update = ups[:limit]
                    for j, up in enumerate(ups[limit:]):
                        nop = mybir.InstNoOp(name=f"{inst.name}-us{j}", ins=[], outs=[])
                        nop.engine = inst.engine
                        nop.sync_info = mybir.SyncInfo(on_wait=[], on_update=[up])
                        nc.register_instruction(nop)
                        new.append(nop)
            bb.instructions[:] = new


def build_nc(s=S, repeat=1):
    st = s // P             # number of 128-wide k tiles
    qc_w = min(512, s)      # q chunk width
    nqc = s // qc_w

    nc = bass.Bass()
    xb = nc.dram_tensor("xb", [P, HT, s], BF16, kind="ExternalInput")
    wqb = nc.dram_tensor("wqb", [P, HT, EW], BF16, kind="ExternalInput")
    wkub = nc.dram_tensor("wkub", [P, HT, HD + GQ], BF16, kind="ExternalInput")
    mkf = nc.dram_tensor("mkf", [P, st], F32, kind="ExternalInput")
    uscr = nc.dram_tensor("uscr", [GQ, s], F32)
    out = nc.dram_tensor("out", [GQ, 2, s], F32, kind="ExternalOutput")

    with tile.TileContext(nc) as tc:
        with (
            tc.tile_pool(name="persist", bufs=1) as persist,
            tc.tile_pool(name="ep", bufs=6) as ep,
            tc.tile_pool(name="obp", bufs=2) as obp,
            tc.tile_pool(name="psum_s", bufs=3, space="PSUM") as psum_s,
            tc.tile_pool(name="psum_o", bufs=2, space="PSUM") as psum_o,
        ):
            xb_sb = persist.tile([P, HT, s], BF16)
            wqb_sb = persist.tile([P, HT, EW], BF16)
            wkub_sb = persist.tile([P, HT, HD + GQ], BF16)
            mkf_sb = persist.tile([P, st], F32)
            KT2 = persist.tile([P, s], BF16)
            QT = [persist.tile([P, s], BF16, tag=f"qt_{i}", name=f"qt_{i}")
                  for i in range(2)]
            MUB = persist.tile([P, st, 2 * GQ], BF16)
            UST = persist.tile([P, st, GQ], F32)   # u scatter staging
            USTF = persist.tile([P, s], F32)     # u psum->sbuf staging (rows 64-67)

            nc.sync.dma_start(out=wqb_sb[:], in_=wqb[:, :, :])
            nc.sync.dma_start(out=wkub_sb[:], in_=wkub[:, :, :])
            nc.sync.dma_start(out=mkf_sb[:], in_=mkf[:, :])
            for q in range(nqc):
                sl = slice(q * qc_w, (q + 1) * qc_w)
                nc.sync.dma_start(out=xb_sb[:, :, sl], in_=xb[:, :, sl])

            for _ in range(repeat):
                # ---- K+U projection ----
                for q in range(nqc):
                    sl = slice(q * qc_w, (q + 1) * qc_w)
                    ps = psum_s.tile([P, 2, qc_w], F32, tag="ps")
                    for t in range(HT):
                        nc.tensor.matmul(
                            ps[0:HD + GQ, 0, :], lhsT=wkub_sb[:, t, :],
                            rhs=xb_sb[:, t, sl],
                            start=(t == 0), stop=(t == HT - 1))
                    # proj casts on ACT: it is idle during the lead-in
                    nc.scalar.copy(KT2[0:HD, sl], ps[0:HD, 0, :])
                    # u rows (64-67) bounce through DRAM for the k-scatter
                    nc.scalar.copy(USTF[HD:HD + GQ, sl],
                                   ps[HD:HD + GQ, 0, :])
                    nc.sync.dma_start(out=uscr[:, sl],
                                      in_=USTF[HD:HD + GQ, sl])
                # duplicate k^T to rows 64-127 (concurrent head pair matmuls)
                nc.sync.dma_start(out=KT2[HD:P, :], in_=KT2[0:HD, :])

                # MUB: u columns (k-scatter to partition-major, mask, cast)
                for j in range(GQ):
                    nc.sync.dma_start(
                        out=UST[:, :, j], in_=uscr[j, :].rearrange("(t p) -> p t", p=P))
                    nc.gpsimd.tensor_tensor(
                        out=MUB[:, :, 2 * j], in0=UST[:, :, j], in1=mkf_sb[:, :],
                        op=OP.mult)
                    nc.gpsimd.tensor_copy(MUB[:, :, 2 * j + 1], mkf_sb[:, :])

                # ---- Q projection (chunk-major so attention can start early)
                for q in range(nqc):
                    sl = slice(q * qc_w, (q + 1) * qc_w)
                    for p2 in range(2):
                        psq = psum_s.tile([P, 2, qc_w], F32, tag="ps")
                        for t in range(HT):
                            nc.tensor.matmul(
                                psq[:, 0, :],
                                lhsT=wqb_sb[:, t, p2 * P:(p2 + 1) * P],
                                rhs=xb_sb[:, t, sl],
                                start=(t == 0), stop=(t == HT - 1))
                        nc.scalar.copy(QT[p2][:, sl], psq[:, 0, :])

                # ---- attention ----
                for q in range(nqc):
                    qsl = slice(q * qc_w, (q + 1) * qc_w)
                    po = psum_o.tile([P, qc_w], F32, tag="po")
                    # init the partitions the col-tiled AV matmuls skip, so
                    # the full-tile ob copy below reads defined data
                    nc.vector.memset(po[:, :], 0.0)
                    prev = None
                    for kt in range(st):
                        ksl = slice(kt * P, (kt + 1) * P)
                        es = []
                        for hp in range(2):
                            ps = psum_s.tile([P, 2, qc_w], F32, tag="ps")
                            nc.tensor.matmul(
                                ps[:, 0, :], lhsT=KT2[0:HD, ksl],
                                rhs=QT[hp][0:HD, qsl], start=True, stop=True,
                                tile_position=(0, 0))
                            nc.tensor.matmul(
                                ps[:, 1, :], lhsT=KT2[HD:P, ksl],
                                rhs=QT[hp][HD:P, qsl], start=True, stop=True,
                                tile_position=(HD, 0))
                            e = ep.tile([P, 2, qc_w], BF16, tag="e")
                            eng = EXP_ASSIGN[(2 * kt + hp) % 16]
                            if eng == "act":
                                nc.scalar.activation(
                                    e[:, :, :], ps[:, :, :], AF.Exp,
                                    scale=1.0 / 8.0)
                            else:
                                nc.vector.tensor_scalar(
                                    out=e[:, :, :].bitcast(U16),
                                    in0=ps[:, :, :], scalar1=SCH_A,
                                    scalar2=SCH_B, op0=OP.mult, op1=OP.add)
                            es.append(e)
                        if prev is not None:
                            pk, pe1, pe2 = prev
                            for h, (pe, c) in enumerate(
                                ((pe1, 0), (pe1, 1), (pe2, 0), (pe2, 1))
                            ):
                                nc.tensor.matmul(
                                    po[32 * h:32 * h + 2, :],
                                    lhsT=MUB[:, pk, 2 * h:2 * h + 2],
                                    rhs=pe[:, c, :],
                                    start=(pk == 0), stop=False,
                                    tile_position=(0, 32 * h))
                        prev = (kt, es[0], es[1])
                    pk, pe1, pe2 = prev
                    for h, (pe, c) in enumerate(
                        ((pe1, 0), (pe1, 1), (pe2, 0), (pe2, 1))
                    ):
                        nc.tensor.matmul(
                            po[32 * h:32 * h + 2, :],
                            lhsT=MUB[:, pk, 2 * h:2 * h + 2],
                            rhs=pe[:, c, :],
                            start=(pk == 0), stop=True,
                            tile_position=(0, 32 * h))
                    ob = obp.tile([P, qc_w], F32, tag="ob")
                    nc.vector.tensor_copy(ob[:, :], po[:, :])
                    for h in range(GQ):
                        nc.sync.dma_start(
                            out=out[h, :, qsl],
                            in_=ob[32 * h:32 * h + 2, :])

    _split_excess_waits(nc)
    return nc


_NC_CACHE = {}


def _get_nc(s=S, repeat=1):
    key = (s, repeat)
    if key not in _NC_CACHE:
        _NC_CACHE[key] = build_nc(s, repeat)
    return _NC_CACHE[key]


def make_inputs(hidden_states, attention_mask, wq, wk, wv, wo, s=S):
    """Host-side shard prep: per-core input dicts (bf16, h-tiled)."""
    hidden_states = np.asarray(hidden_states, dtype=np.float32)
    attention_mask = np.asarray(attention_mask)
    wq = np.asarray(wq, dtype=np.float32)
    wk = np.asarray(wk, dtype=np.float32)
    wv = np.asarray(wv, dtype=np.float32)
    wo = np.asarray(wo, dtype=np.float32)
    st = s // P

    def h_pack(a):
        # [H, C] -> [128, HT, C]  with h = 128*t + j
        c = a.shape[1]
        return np.ascontiguousarray(a.reshape(HT, P, c).transpose(1, 0, 2))

    in_maps = []
    for core in range(NCORES):
        b, g = divmod(core, NKV)
        xT = np.ascontiguousarray(hidden_states[b, :s, :].T)      # [H, s]
        xb = h_pack(xT).astype(NPBF)
        wq_g = wq[:, g * EW:(g + 1) * EW]
        wqb = h_pack(wq_g).astype(NPBF)
        wk_g = wk[:, g * HD:(g + 1) * HD]
        wo_g = wo[g * EW:(g + 1) * EW, 0].reshape(GQ, HD).T        # [HD, GQ]
        wu_g = wv[:, g * HD:(g + 1) * HD] @ wo_g
        wkub = h_pack(np.concatenate([wk_g, wu_g], axis=1)).astype(NPBF)
        m = (attention_mask[b, :s] != 0).astype(np.float32)
        mkf = np.ascontiguousarray(m.reshape(st, P).T)             # [128, st]
        in_maps.append({"xb": xb, "wqb": wqb, "wkub": wkub, "mkf": mkf})
    return in_maps


def combine(results, s=S):
    """Host-side gather: out[b,q] = sum over group cores and heads num/den."""
    out = np.zeros((B, s, 1), dtype=np.float32)
    for core in range(NCORES):
        b = core // NKV
        nd = results[core]["out"]          # [GQ, 2, s]
        out[b, :, 0] += (nd[:, 0, :] / nd[:, 1, :]).sum(axis=0)
    return out


def kernel(hidden_states, attention_mask, wq, wk, wv, wo):
    nc = _get_nc()
    in_maps = make_inputs(hidden_states, attention_mask, wq, wk, wv, wo)
    res = run_bass_kernel_spmd(nc, in_maps, core_ids=list(range(NCORES)))
    return combine(res.results)


# revision 4
# speedup vs baseline: 1.3078x; 1.0246x over previous
"""Trainium2 Bass kernel for CoEncoderDynamicAttention (v3: col-tiled AV,
1-bank PSUM accumulator, rebalanced exp split).

Model (reference):
  q = x @ wq   -> [B,S,NH,HD];  k = x @ wk -> [B,S,NKV,HD];  v = x @ wv
  scores = q k^T / sqrt(HD), masked, softmax over k
  out = (attn @ v) reshaped @ wo        (wo: [NH*HD, 1])

Sharding: 8 cores = (batch b in 0..1) x (kv-group g in 0..3).  Each kv
group owns 1 kv head and GQ=4 q heads.  Since wo has output dim 1, fold
wo into v on the host:  u_h = v_g @ wo_h, so per-core output is
  num_h[q] = sum_k m[k] u_h[k] e_h[k,q],  den_h[q] = sum_k m[k] e_h[k,q]
with e = exp(s/sqrt(HD)); the mask is folded multiplicatively into the
AV stationary operand.  Host combines out[b,q] = sum_{g,h} num_h/den_h.

Perf structure vs v2:
  * Scores matmuls pair heads in disjoint PE row groups (KT duplicated to
    partitions 64-127), both pairs (heads 0,1 then 2,3) per k-tile.
  * AV matmuls (M=2: num/den) are 4x COLUMN-TILED: head h ->
    tile_position (0, 32h), output partitions 32h..32h+1 of ONE 1-bank
    PSUM accumulator.  All four stream concurrently on disjoint PE
    column groups, so AV costs ~512 cycles/k-tile instead of ~2048.
  * exp split ACT/DVE tuned so both engines finish together; projection
    casts go to ACT (idle during the projection lead-in anyway).
  * AV for k-tile t is emitted after the scores matmuls of k-tile t+1 so
    the tensor queue never waits on the exp engines.

Precision: all matmuls bf16 (operand noise passes straight to the output
through the cancelling num sums).  DVE share of exp uses the Schraudolph
bit trick in bf16: u16 = round(ps*A + B) written as uint16 IS the bf16
encoding of ~exp(s) (~2% rms sawtooth, diluted across tiles).
"""

import numpy as np
import ml_dtypes

import concourse.bass as bass
import concourse.mybir as mybir
import concourse.tile as tile
from concourse.bass_utils import run_bass_kernel_spmd

B, S, H = 2, 2048, 1024
NH, NKV, HD = 16, 4, 64
GQ = NH // NKV          # q heads per kv group
EW = GQ * HD            # per-core q projection width (256)
NCORES = 8
P = 128
HT = H // P             # h (contraction) tiles
F32 = mybir.dt.float32
BF16 = mybir.dt.bfloat16
U16 = mybir.dt.uint16
AF = mybir.ActivationFunctionType
OP = mybir.AluOpType
NPBF = ml_dtypes.bfloat16

# Schraudolph-to-bf16: for psum score ps (= 8*s_true),
# u16 = round(ps*SCH_A + SCH_B) is the bf16 bit pattern of ~exp(s_true).
SCH_A = float(128.0 * np.log2(np.e) / 8.0)
SCH_B = float(16256.0 - 7.373)

# per-exp-tile engine pick, indexed by (2*kt + pair) % 16: strict 1:1
# alternation keeps both engines streaming with the 3-deep PSUM pipeline
# (ACT: ~1049ns/tile, DVE: ~1202ns/tile, ACT carries the projection casts)
EXP_ASSIGN = ["act", "dve"] * 8


def _split_excess_waits(nc, limit=1):
    """This walrus build only accepts one sync-wait (and update) per
    instruction; hoist extras onto NoOps on the same engine."""
    for f in nc.m.functions:
        for bb in f.blocks:
            new = []
            for inst in bb.instructions:
                si = getattr(inst, "sync_info", None)
                waits = list(si.on_wait) if (si is not None and si.on_wait) else []
                k = 0
                while len(waits) > limit:
                    chunk, waits = waits[:limit], waits[limit:]
                    nop = mybir.InstNoOp(name=f"{inst.name}-ws{k}", ins=[], outs=[])
                    nop.engine = inst.engine
                    nop.sync_info = mybir.SyncInfo(on_wait=chunk, on_update=[])
                    nc.register_instruction(nop)
                    new.append(nop)
                    k += 1
                if k:
                    si.on_wait = waits
                new.append(inst)
                ups = list(si.on_update) if (si is not None and si.on_update) else []
                if len(ups) > limit and type(inst).__name__ not in (
                    "InstDMA", "InstDMACopy", "InstTensorLoad", "InstTensorSave",
                ):
                    si.on_update = ups[:limit]
                    for j, up in enumerate(ups[limit:]):
                        nop = mybir.InstNoOp(name=f"{inst.name}-us{j}", ins=[], outs=[])
                        nop.engine = inst.engine
                        nop.sync_info = mybir.SyncInfo(on_wait=[], on_update=[up])
                        nc.register_instruction(nop)
                        new.append(nop)
            bb.instructions[:] = new


def build_nc(s=S, repeat=1):
    st = s // P             # number of 128-wide k tiles
    qc_w = min(512, s)      # q chunk width
    nqc = s // qc_w

    nc = bass.Bass()
    xb = nc.dram_tensor("xb", [P, HT, s], BF16, kind="ExternalInput")
    wqb = nc.dram_tensor("wqb", [P, HT, EW], BF16, kind="ExternalInput")
    wkub = nc.dram_tensor("wkub", [P, HT, HD + GQ], BF16, kind="ExternalInput")
    mkf = nc.dram_tensor("mkf", [P, st], F32, kind="ExternalInput")
    uscr = nc.dram_tensor("uscr", [GQ, s], F32)
    out = nc.dram_tensor("out", [GQ, 2, s], F32, kind="ExternalOutput")

    with tile.TileContext(nc) as tc:
        with (
            tc.tile_pool(name="persist", bufs=1) as persist,
            tc.tile_pool(name="ep", bufs=6) as ep,
            tc.tile_pool(name="obp", bufs=2) as obp,
            tc.tile_pool(name="psum_s", bufs=3, space="PSUM") as psum_s,
            tc.tile_pool(name="psum_o", bufs=2, space="PSUM") as psum_o,
        ):
            xb_sb = persist.tile([P, HT, s], BF16)
            wqb_sb = persist.tile([P, HT, EW], BF16)
            wkub_sb = persist.tile([P, HT, HD + GQ], BF16)
            mkf_sb = persist.tile([P, st], F32)
            KT2 = persist.tile([P, s], BF16)
            QT = [persist.tile([P, s], BF16, tag=f"qt_{i}", name=f"qt_{i}")
                  for i in range(2)]
            MUB = persist.tile([P, st, 2 * GQ], BF16)
            UST = persist.tile([P, st, GQ], F32)   # u scatter staging
            USTF = persist.tile([P, s], F32)     # u psum->sbuf staging (rows 64-67)

            nc.sync.dma_start(out=wqb_sb[:], in_=wqb[:, :, :])
            nc.sync.dma_start(out=wkub_sb[:], in_=wkub[:, :, :])
            nc.sync.dma_start(out=mkf_sb[:], in_=mkf[:, :])
            for q in range(nqc):
                sl = slice(q * qc_w, (q + 1) * qc_w)
                nc.sync.dma_start(out=xb_sb[:, :, sl], in_=xb[:, :, sl])

            for _ in range(repeat):
                # ---- K+U projection ----
                for q in range(nqc):
                    sl = slice(q * qc_w, (q + 1) * qc_w)
                    ps = psum_s.tile([P, 2, qc_w], F32, tag="ps")
                    for t in range(HT):
                        nc.tensor.matmul(
                            ps[0:HD + GQ, 0, :], lhsT=wkub_sb[:, t, :],
                            rhs=xb_sb[:, t, sl],
                            start=(t == 0), stop=(t == HT - 1))
                    # proj casts on ACT: it is idle during the lead-in
                    nc.scalar.copy(KT2[0:HD, sl], ps[0:HD, 0, :])
                    # u rows (64-67) bounce through DRAM for the k-scatter
                    nc.scalar.copy(USTF[HD:HD + GQ, sl],
                                   ps[HD:HD + GQ, 0, :])
                    nc.sync.dma_start(out=uscr[:, sl],
                                      in_=USTF[HD:HD + GQ, sl])
                # duplicate k^T to rows 64-127 (concurrent head pair matmuls)
                nc.sync.dma_start(out=KT2[HD:P, :], in_=KT2[0:HD, :])

                # MUB: u columns (k-scatter to partition-major, mask, cast)
                for j in range(GQ):
                    nc.sync.dma_start(
                        out=UST[:, :, j], in_=uscr[j, :].rearrange("(t p) -> p t", p=P))
                    nc.gpsimd.tensor_tensor(
                        out=MUB[:, :, 2 * j], in0=UST[:, :, j], in1=mkf_sb[:, :],
                        op=OP.mult)
                    nc.gpsimd.tensor_copy(MUB[:, :, 2 * j + 1], mkf_sb[:, :])

                # ---- Q projection (chunk-major so attention can start early)
                for q in range(nqc):
                    sl = slice(q * qc_w, (q + 1) * qc_w)
                    for p2 in range(2):
                        psq = psum_s.tile([P, 2, qc_w], F32, tag="ps")
                        for t in range(HT):
                            nc.tensor.matmul(
                                psq[:, 0, :],
                                lhsT=wqb_sb[:, t, p2 * P:(p2 + 1) * P],
                                rhs=xb_sb[:, t, sl],
                                start=(t == 0), stop=(t == HT - 1))
                        nc.scalar.copy(QT[p2][:, sl], psq[:, 0, :])

                # ---- attention ----
                for q in range(nqc):
                    qsl = slice(q * qc_w, (q + 1) * qc_w)
                    po = psum_o.tile([P, qc_w], F32, tag="po")
                    # init the partitions the col-tiled AV matmuls skip, so
                    # the full-tile ob copy below reads defined data
                    nc.vector.memset(po[:, :], 0.0)
                    prev = None
                    for kt in range(st):
                        ksl = slice(kt * P, (kt + 1) * P)
                        es = []
                        for hp in range(2):
                            ps = psum_s.tile([P, 2, qc_w], F32, tag="ps")
                            nc.tensor.matmul(
                                ps[:, 0, :], lhsT=KT2[0:HD, ksl],
                                rhs=QT[hp][0:HD, qsl], start=True, stop=True,
                                tile_position=(0, 0))
                            nc.tensor.matmul(
                                ps[:, 1, :], lhsT=KT2[HD:P, ksl],
                                rhs=QT[hp][HD:P, qsl], start=True, stop=True,
                                tile_position=(HD, 0))
                            e = ep.tile([P, 2, qc_w], BF16, tag="e")
                            eng = EXP_ASSIGN[(2 * kt + hp) % 16]
                            if eng == "act":
                                nc.scalar.activation(
                                    e[:, :, :], ps[:, :, :], AF.Exp,
                                    scale=1.0 / 8.0)
                            else:
                                nc.vector.tensor_scalar(
                                    out=e[:, :, :].bitcast(U16),
                                    in0=ps[:, :, :], scalar1=SCH_A,
                                    scalar2=SCH_B, op0=OP.mult, op1=OP.add)
                            es.append(e)
                        if prev is not None:
                            pk, pe1, pe2 = prev
                            for h, (pe, c) in enumerate(
                                ((pe1, 0), (pe1, 1), (pe2, 0), (pe2, 1))
                            ):
                                nc.tensor.matmul(
                                    po[32 * h:32 * h + 2, :],
                                    lhsT=MUB[:, pk, 2 * h:2 * h + 2],
                                    rhs=pe[:, c, :],
                                    start=(pk == 0), stop=False,
                                    tile_position=(0, 32 * h))
                        prev = (kt, es[0], es[1])
                    pk, pe1, pe2 = prev
                    for h, (pe, c) in enumerate(
                        ((pe1, 0), (pe1, 1), (pe2, 0), (pe2, 1))
                    ):
                        nc.tensor.matmul(
                            po[32 * h:32 * h + 2, :],
                            lhsT=MUB[:, pk, 2 * h:2 * h + 2],
                            rhs=pe[:, c, :],
                            start=(pk == 0), stop=True,
                            tile_position=(0, 32 * h))
                    ob = obp.tile([P, qc_w], F32, tag="ob")
                    nc.vector.tensor_copy(ob[:, :], po[:, :])
                    for h in range(GQ):
                        nc.sync.dma_start(
                            out=out[h, :, qsl],
                            in_=ob[32 * h:32 * h + 2, :])

    _split_excess_waits(nc)
    return nc


_NC_CACHE = {}


def _get_nc(s=S, repeat=1):
    key = (s, repeat)
    if key not in _NC_CACHE:
        _NC_CACHE[key] = build_nc(s, repeat)
    return _NC_CACHE[key]


def make_inputs(hidden_states, attention_mask, wq, wk, wv, wo, s=S):
    """Host-side shard prep: per-core input dicts (bf16, h-tiled)."""
    hidden_states = np.asarray(hidden_states, dtype=np.float32)
    attention_mask = np.asarray(attention_mask)
    wq = np.asarray(wq, dtype=np.float32)
    wk = np.asarray(wk, dtype=np.float32)
    wv = np.asarray(wv, dtype=np.float32)
    wo = np.asarray(wo, dtype=np.float32)
    st = s // P

    def h_pack(a):
        # [H, C] -> [128, HT, C]  with h = 128*t + j
        c = a.shape[1]
        return np.ascontiguousarray(a.reshape(HT, P, c).transpose(1, 0, 2))

    in_maps = []
    for core in range(NCORES):
        b, g = divmod(core, NKV)
        xT = np.ascontiguousarray(hidden_states[b, :s, :].T)      # [H, s]
        xb = h_pack(xT).astype(NPBF)
        wq_g = wq[:, g * EW:(g + 1) * EW]
        wqb = h_pack(wq_g).astype(NPBF)
        wk_g = wk[:, g * HD:(g + 1) * HD]
        wo_g = wo[g * EW:(g + 1) * EW, 0].reshape(GQ, HD).T        # [HD, GQ]
        wu_g = wv[:, g * HD:(g + 1) * HD] @ wo_g
        wkub = h_pack(np.concatenate([wk_g, wu_g], axis=1)).astype(NPBF)
        m = (attention_mask[b, :s] != 0).astype(np.float32)
        mkf = np.ascontiguousarray(m.reshape(st, P).T)             # [128, st]
        in_maps.append({"xb": xb, "wqb": wqb, "wkub": wkub, "mkf": mkf})
    return in_maps


def combine(results, s=S):
    """Host-side gather: out[b,q] = sum over group cores and heads num/den."""
    out = np.zeros((B, s, 1), dtype=np.float32)
    for core in range(NCORES):
        b = core // NKV
        nd = results[core]["out"]          # [GQ, 2, s]
        out[b, :, 0] += (nd[:, 0, :] / nd[:, 1, :]).sum(axis=0)
    return out


def kernel(hidden_states, attention_mask, wq, wk, wv, wo):
    nc = _get_nc()
    in_maps = make_inputs(hidden_states, attention_mask, wq, wk, wv, wo)
    res = run_bass_kernel_spmd(nc, in_maps, core_ids=list(range(NCORES)))
    return combine(res.results)


# revision 7
# speedup vs baseline: 1.3370x; 1.0223x over previous
"""Trainium2 Bass kernel for CoEncoderDynamicAttention (v5: col-tiled AV,
cross-iteration software pipelining, balanced ACT/DVE exp).

Model (reference):
  q = x @ wq   -> [B,S,NH,HD];  k = x @ wk -> [B,S,NKV,HD];  v = x @ wv
  scores = q k^T / sqrt(HD), masked, softmax over k
  out = (attn @ v) reshaped @ wo        (wo: [NH*HD, 1])

Sharding: 8 cores = (batch b in 0..1) x (kv-group g in 0..3).  Each kv
group owns 1 kv head and GQ=4 q heads.  Since wo has output dim 1, fold
wo into v on the host:  u_h = v_g @ wo_h, so per-core output is
  num_h[q] = sum_k m[k] u_h[k] e_h[k,q],  den_h[q] = sum_k m[k] e_h[k,q]
with e = exp(s/sqrt(HD)); the mask is folded multiplicatively into the
AV stationary operand.  Host combines out[b,q] = sum_{g,h} num_h/den_h.

Perf structure (the kernel is exp-bound: every scores element must leave
PSUM through ACT or DVE, ~277G elem/s total):
  * Scores matmuls pair heads in disjoint PE row groups (KT duplicated
    to partitions 64-127); both pairs (heads 0,1 / 2,3) per k-tile.
  * AV matmuls (M=2: num/den) are 4x COLUMN-TILED: head h ->
    tile_position (0, 32h), output partitions 32h..32h+1 of ONE 1-bank
    PSUM accumulator; all four stream concurrently.
  * exp strictly alternates ACT (native Exp) / DVE (Schraudolph-to-bf16
    bit trick) so both engines stream continuously.
  * Cross-iteration pipelining: KT2/QT/MUB/uscr are double-buffered by
    iteration parity and the NEXT iteration's K/Q projection matmul
    groups are emitted interleaved (one per 4 k-tiles) inside the
    current iteration's attention, filling the PE's idle slots, so the
    exp engines never wait for projections at iteration boundaries.
  * The per-q-chunk output copy is emitted two k-tiles INTO the next
    chunk so it never blocks the DVE exp stream at chunk boundaries.
  * AV for k-tile t is emitted after the scores matmuls of k-tile t+1
    so the tensor queue never waits on the exp engines.

PSUM budget (8 banks): 3 x 2-bank score tiles + 1-bank projection tile
+ 1-bank AV accumulator = 8.
"""

import numpy as np
import ml_dtypes

import concourse.bass as bass
import concourse.mybir as mybir
import concourse.tile as tile
from concourse.bass_utils import run_bass_kernel_spmd

B, S, H = 2, 2048, 1024
NH, NKV, HD = 16, 4, 64
GQ = NH // NKV          # q heads per kv group
EW = GQ * HD            # per-core q projection width (256)
NCORES = 8
P = 128
HT = H // P             # h (contraction) tiles
F32 = mybir.dt.float32
BF16 = mybir.dt.bfloat16
U16 = mybir.dt.uint16
AF = mybir.ActivationFunctionType
OP = mybir.AluOpType
NPBF = ml_dtypes.bfloat16

# Schraudolph-to-bf16: for psum score ps (= 8*s_true),
# u16 = round(ps*SCH_A + SCH_B) is the bf16 bit pattern of ~exp(s_true).
SCH_A = float(128.0 * np.log2(np.e) / 8.0)
SCH_B = float(16256.0 - 7.373)


def _split_excess_waits(nc, limit=1):
    """This walrus build only accepts one sync-wait (and update) per
    instruction; hoist extras onto NoOps on the same engine."""
    for f in nc.m.functions:
        for bb in f.blocks:
            new = []
            for inst in bb.instructions:
                si = getattr(inst, "sync_info", None)
                waits = list(si.on_wait) if (si is not None and si.on_wait) else []
                k = 0
                while len(waits) > limit:
                    chunk, waits = waits[:limit], waits[limit:]
                    nop = mybir.InstNoOp(name=f"{inst.name}-ws{k}", ins=[], outs=[])
                    nop.engine = inst.engine
                    nop.sync_info = mybir.SyncInfo(on_wait=chunk, on_update=[])
                    nc.register_instruction(nop)
                    new.append(nop)
                    k += 1
                if k:
                    si.on_wait = waits
                new.append(inst)
                ups = list(si.on_update) if (si is not None and si.on_update) else []
                if len(ups) > limit and type(inst).__name__ not in (
                    "InstDMA", "InstDMACopy", "InstTensorLoad", "InstTensorSave",
                ):
                    si.on_update = ups[:limit]
                    for j, up in enumerate(ups[limit:]):
                        nop = mybir.InstNoOp(name=f"{inst.name}-us{j}", ins=[], outs=[])
                        nop.engine = inst.engine
                        nop.sync_info = mybir.SyncInfo(on_wait=[], on_update=[up])
                        nc.register_instruction(nop)
                        new.append(nop)
            bb.instructions[:] = new


def build_nc(s=S, repeat=1):
    st = s // P             # number of 128-wide k tiles
    qc_w = min(512, s)      # q chunk width
    nqc = s // qc_w

    nc = bass.Bass()
    xb = nc.dram_tensor("xb", [P, HT, s], BF16, kind="ExternalInput")
    wqb = nc.dram_tensor("wqb", [P, HT, EW], BF16, kind="ExternalInput")
    wkub = nc.dram_tensor("wkub", [P, HT, HD + GQ], BF16, kind="ExternalInput")
    mkf = nc.dram_tensor("mkf", [P, st], F32, kind="ExternalInput")
    uscr = [nc.dram_tensor(f"uscr{i}", [GQ, s], BF16) for i in range(2)]
    out = nc.dram_tensor("out", [GQ, 2, s], F32, kind="ExternalOutput")

    with tile.TileContext(nc) as tc:
        with (
            tc.tile_pool(name="persist", bufs=1) as persist,
            tc.tile_pool(name="ep", bufs=6) as ep,
            tc.tile_pool(name="obp", bufs=2) as obp,
            tc.tile_pool(name="psum_s", bufs=3, space="PSUM") as psum_s,
            tc.tile_pool(name="psum_p", bufs=1, space="PSUM") as psum_p,
            tc.tile_pool(name="psum_o", bufs=1, space="PSUM") as psum_o,
        ):
            xb_sb = persist.tile([P, HT, s], BF16)
            wqb_sb = persist.tile([P, HT, EW], BF16)
            wkub_sb = persist.tile([P, HT, HD + GQ], BF16)
            mkf_sb = persist.tile([P, st], F32)
            # per-iteration-parity projection outputs
            KT2 = [persist.tile([P, s], BF16, tag=f"kt2_{i}", name=f"kt2_{i}")
                   for i in range(2)]
            QT = [[persist.tile([P, s], BF16, tag=f"qt_{i}_{p}", name=f"qt_{i}_{p}")
                   for p in range(2)] for i in range(2)]
            KTU = [persist.tile([P, s], BF16, tag=f"ktu_{i}", name=f"ktu_{i}")
                   for i in range(2)]
            MUB = [persist.tile([P, st, 2 * GQ], BF16, tag=f"mub_{i}",
                                name=f"mub_{i}") for i in range(2)]
            UST = [persist.tile([P, st, GQ], BF16, tag=f"ust_{i}",
                                name=f"ust_{i}") for i in range(2)]

            nc.sync.dma_start(out=wqb_sb[:], in_=wqb[:, :, :])
            nc.sync.dma_start(out=wkub_sb[:], in_=wkub[:, :, :])
            nc.sync.dma_start(out=mkf_sb[:], in_=mkf[:, :])
            for q in range(nqc):
                sl = slice(q * qc_w, (q + 1) * qc_w)
                nc.sync.dma_start(out=xb_sb[:, :, sl], in_=xb[:, :, sl])

            def emit_proj_group(par, gi):
                """One projection matmul group for iteration-parity `par`.
                gi 0..3: K+U chunk gi; gi 4..11: Q chunk (gi-4)//2 half
                (gi-4)%2; gi 12: the MUB u-scatter."""
                if gi < 4:
                    sl = slice(gi * qc_w, (gi + 1) * qc_w)
                    psp = psum_p.tile([P, qc_w], F32, tag="psp")
                    for t in range(HT):
                        nc.tensor.matmul(
                            psp[0:HD + GQ, :], lhsT=wkub_sb[:, t, :],
                            rhs=xb_sb[:, t, sl],
                            start=(t == 0), stop=(t == HT - 1))
                    # single merged cast (k rows 0-63 + u rows 64-67); the
                    # fan-out below is DMA-only
                    nc.scalar.copy(KTU[par][0:HD + GQ, sl], psp[0:HD + GQ, :])
                    nc.sync.dma_start(out=KT2[par][0:HD, sl],
                                      in_=KTU[par][0:HD, sl])
                    nc.sync.dma_start(out=KT2[par][HD:P, sl],
                                      in_=KTU[par][0:HD, sl])
                    nc.sync.dma_start(out=uscr[par][:, sl],
                                      in_=KTU[par][HD:HD + GQ, sl])
                elif gi < 12:
                    q, p2 = (gi - 4) // 2, (gi - 4) % 2
                    sl = slice(q * qc_w, (q + 1) * qc_w)
                    psp = psum_p.tile([P, qc_w], F32, tag="psp")
                    for t in range(HT):
                        nc.tensor.matmul(
                            psp[:, :],
                            lhsT=wqb_sb[:, t, p2 * P:(p2 + 1) * P],
                            rhs=xb_sb[:, t, sl],
                            start=(t == 0), stop=(t == HT - 1))
                    nc.scalar.copy(QT[par][p2][:, sl], psp[:, :])
                else:
                    # u columns: k-scatter to partition-major, mask, cast
                    for j in range(GQ):
                        nc.sync.dma_start(
                            out=UST[par][:, :, j],
                            in_=uscr[par][j, :].rearrange("(t p) -> p t", p=P))
                        nc.gpsimd.tensor_tensor(
                            out=MUB[par][:, :, 2 * j], in0=UST[par][:, :, j],
                            in1=mkf_sb[:, :], op=OP.mult)
                        nc.gpsimd.tensor_copy(MUB[par][:, :, 2 * j + 1],
                                              mkf_sb[:, :])

            def emit_out_drain(pend):
                """po -> SBUF -> DRAM for a finished q chunk."""
                po, qsl = pend
                ob = obp.tile([P, qc_w], F32, tag="ob")
                nc.vector.tensor_copy(ob[:, :], po[:, :])
                nc.vector.memset(next_po[0][:, :], 0.0)
                for h in range(GQ):
                    nc.sync.dma_start(
                        out=out[h, :, qsl], in_=ob[32 * h:32 * h + 2, :])

            # iteration 0's projections run up front (lead-in, paid once)
            for gi in range(13):
                emit_proj_group(0, gi)

            pending = None          # (po, qsl) of the q chunk awaiting drain
            next_po = [None]        # po tile whose memset must follow the drain

            for it in range(repeat):
                cur = it % 2
                nxt = (it + 1) % 2
                emit_next = it + 1 < repeat
                for q in range(nqc):
                    qsl = slice(q * qc_w, (q + 1) * qc_w)
                    po = psum_o.tile([P, qc_w], F32, tag="po")
                    if pending is None:
                        # very first chunk: nothing to drain, memset directly
                        nc.vector.memset(po[:, :], 0.0)
                    next_po[0] = po
                    prev = None
                    for kt in range(st):
                        ksl = slice(kt * P, (kt + 1) * P)
                        es = []
                        for hp in range(2):
                            ps = psum_s.tile([P, 2, qc_w], F32, tag="ps")
                            nc.tensor.matmul(
                                ps[:, 0, :], lhsT=KT2[cur][0:HD, ksl],
                                rhs=QT[cur][hp][0:HD, qsl], start=True,
                                stop=True, tile_position=(0, 0))
                            nc.tensor.matmul(
                                ps[:, 1, :], lhsT=KT2[cur][HD:P, ksl],
                                rhs=QT[cur][hp][HD:P, qsl], start=True,
                                stop=True, tile_position=(HD, 0))
                            e = ep.tile([P, 2, qc_w], BF16, tag="e")
                            if hp == 0:
                                nc.scalar.activation(
                                    e[:, :, :], ps[:, :, :], AF.Exp,
                                    scale=1.0 / 8.0)
                            else:
                                nc.vector.tensor_scalar(
                                    out=e[:, :, :].bitcast(U16),
                                    in0=ps[:, :, :], scalar1=SCH_A,
                                    scalar2=SCH_B, op0=OP.mult, op1=OP.add)
                            es.append(e)
                        # drain the previous chunk's accumulator before this
                        # chunk's first AV flush: the memset (same 1-bank
                        # pool slot) must precede AV(kt0) on the DVE queue
                        # or the PE blocks behind an exp it hasn't fed yet
                        if kt == 0 and pending is not None:
                            emit_out_drain(pending)
                            pending = None
                        if prev is not None:
                            pk, pe1, pe2 = prev
                            for h, (pe, c) in enumerate(
                                ((pe1, 0), (pe1, 1), (pe2, 0), (pe2, 1))
                            ):
                                nc.tensor.matmul(
                                    po[32 * h:32 * h + 2, :],
                                    lhsT=MUB[cur][:, pk, 2 * h:2 * h + 2],
                                    rhs=pe[:, c, :],
                                    start=(pk == 0), stop=False,
                                    tile_position=(0, 32 * h))
                        prev = (kt, es[0], es[1])
                        if emit_next and kt % 4 == 3:
                            gi = q * 4 + kt // 4
                            if gi < 13:
                                emit_proj_group(nxt, gi)
                    pk, pe1, pe2 = prev
                    for h, (pe, c) in enumerate(
                        ((pe1, 0), (pe1, 1), (pe2, 0), (pe2, 1))
                    ):
                        nc.tensor.matmul(
                            po[32 * h:32 * h + 2, :],
                            lhsT=MUB[cur][:, pk, 2 * h:2 * h + 2],
                            rhs=pe[:, c, :],
                            start=(pk == 0), stop=True,
                            tile_position=(0, 32 * h))
                    pending = (po, qsl)

            # final drain (no successor chunk)
            po, qsl = pending
            ob = obp.tile([P, qc_w], F32, tag="ob")
            nc.vector.tensor_copy(ob[:, :], po[:, :])
            for h in range(GQ):
                nc.sync.dma_start(
                    out=out[h, :, qsl], in_=ob[32 * h:32 * h + 2, :])

    _split_excess_waits(nc)
    return nc


_NC_CACHE = {}


def _get_nc(s=S, repeat=1):
    key = (s, repeat)
    if key not in _NC_CACHE:
        _NC_CACHE[key] = build_nc(s, repeat)
    return _NC_CACHE[key]


def make_inputs(hidden_states, attention_mask, wq, wk, wv, wo, s=S):
    """Host-side shard prep: per-core input dicts (bf16, h-tiled)."""
    hidden_states = np.asarray(hidden_states, dtype=np.float32)
    attention_mask = np.asarray(attention_mask)
    wq = np.asarray(wq, dtype=np.float32)
    wk = np.asarray(wk, dtype=np.float32)
    wv = np.asarray(wv, dtype=np.float32)
    wo = np.asarray(wo, dtype=np.float32)
    st = s // P

    def h_pack(a):
        # [H, C] -> [128, HT, C]  with h = 128*t + j
        c = a.shape[1]
        return np.ascontiguousarray(a.reshape(HT, P, c).transpose(1, 0, 2))

    in_maps = []
    for core in range(NCORES):
        b, g = divmod(core, NKV)
        xT = np.ascontiguousarray(hidden_states[b, :s, :].T)      # [H, s]
        xb = h_pack(xT).astype(NPBF)
        wq_g = wq[:, g * EW:(g + 1) * EW]
        wqb = h_pack(wq_g).astype(NPBF)
        wk_g = wk[:, g * HD:(g + 1) * HD]
        wo_g = wo[g * EW:(g + 1) * EW, 0].reshape(GQ, HD).T        # [HD, GQ]
        wu_g = wv[:, g * HD:(g + 1) * HD] @ wo_g
        wkub = h_pack(np.concatenate([wk_g, wu_g], axis=1)).astype(NPBF)
        m = (attention_mask[b, :s] != 0).astype(np.float32)
        mkf = np.ascontiguousarray(m.reshape(st, P).T)             # [128, st]
        in_maps.append({"xb": xb, "wqb": wqb, "wkub": wkub, "mkf": mkf})
    return in_maps


def combine(results, s=S):
    """Host-side gather: out[b,q] = sum over group cores and heads num/den."""
    out = np.zeros((B, s, 1), dtype=np.float32)
    for core in range(NCORES):
        b = core // NKV
        nd = results[core]["out"]          # [GQ, 2, s]
        out[b, :, 0] += (nd[:, 0, :] / nd[:, 1, :]).sum(axis=0)
    return out


def kernel(hidden_states, attention_mask, wq, wk, wv, wo):
    nc = _get_nc()
    in_maps = make_inputs(hidden_states, attention_mask, wq, wk, wv, wo)
    res = run_bass_kernel_spmd(nc, in_maps, core_ids=list(range(NCORES)))
    return combine(res.results)


# revision 10
# speedup vs baseline: 1.4924x; 1.1162x over previous
"""Trainium2 Bass kernel for CoEncoderDynamicAttention (v5: col-tiled AV,
cross-iteration software pipelining, balanced ACT/DVE exp).

Model (reference):
  q = x @ wq   -> [B,S,NH,HD];  k = x @ wk -> [B,S,NKV,HD];  v = x @ wv
  scores = q k^T / sqrt(HD), masked, softmax over k
  out = (attn @ v) reshaped @ wo        (wo: [NH*HD, 1])

Sharding: 8 cores = (batch b in 0..1) x (kv-group g in 0..3).  Each kv
group owns 1 kv head and GQ=4 q heads.  Since wo has output dim 1, fold
wo into v on the host:  u_h = v_g @ wo_h, so per-core output is
  num_h[q] = sum_k m[k] u_h[k] e_h[k,q],  den_h[q] = sum_k m[k] e_h[k,q]
with e = exp(s/sqrt(HD)); the mask is folded multiplicatively into the
AV stationary operand.  Host combines out[b,q] = sum_{g,h} num_h/den_h.

Perf structure (the kernel is exp-bound: every scores element must leave
PSUM through ACT or DVE, ~277G elem/s total):
  * Scores matmuls pair heads in disjoint PE row groups (KT duplicated
    to partitions 64-127); both pairs (heads 0,1 / 2,3) per k-tile.
  * AV matmuls (M=2: num/den) are 4x COLUMN-TILED: head h ->
    tile_position (0, 32h), output partitions 32h..32h+1 of ONE 1-bank
    PSUM accumulator; all four stream concurrently.
  * exp strictly alternates ACT (native Exp) / DVE (Schraudolph-to-bf16
    bit trick) so both engines stream continuously.
  * Cross-iteration pipelining: KT2/QT/MUB/uscr are double-buffered by
    iteration parity and the NEXT iteration's K/Q projection matmul
    groups are emitted interleaved (one per 4 k-tiles) inside the
    current iteration's attention, filling the PE's idle slots, so the
    exp engines never wait for projections at iteration boundaries.
  * The per-q-chunk output copy is emitted two k-tiles INTO the next
    chunk so it never blocks the DVE exp stream at chunk boundaries.
  * AV for k-tile t is emitted after the scores matmuls of k-tile t+1
    so the tensor queue never waits on the exp engines.

PSUM budget (8 banks): 3 x 2-bank score tiles + 1-bank projection tile
+ 1-bank AV accumulator = 8.
"""

import numpy as np
import ml_dtypes

import concourse.bass as bass
import concourse.mybir as mybir
import concourse.tile as tile
from concourse.bass_utils import run_bass_kernel_spmd

B, S, H = 2, 2048, 1024
NH, NKV, HD = 16, 4, 64
GQ = NH // NKV          # q heads per kv group
EW = GQ * HD            # per-core q projection width (256)
NCORES = 8
P = 128
HT = H // P             # h (contraction) tiles
F32 = mybir.dt.float32
BF16 = mybir.dt.bfloat16
U16 = mybir.dt.uint16
AF = mybir.ActivationFunctionType
OP = mybir.AluOpType
NPBF = ml_dtypes.bfloat16

# Schraudolph-to-bf16: for psum score ps (= 8*s_true),
# u16 = round(ps*SCH_A + SCH_B) is the bf16 bit pattern of ~exp(s_true).
SCH_A = float(128.0 * np.log2(np.e) / 8.0)
SCH_B = float(16256.0 - 7.373)


def _split_excess_waits(nc, limit=1):
    """This walrus build only accepts one sync-wait (and update) per
    instruction; hoist extras onto NoOps on the same engine."""
    for f in nc.m.functions:
        for bb in f.blocks:
            new = []
            for inst in bb.instructions:
                si = getattr(inst, "sync_info", None)
                waits = list(si.on_wait) if (si is not None and si.on_wait) else []
                k = 0
                while len(waits) > limit:
                    chunk, waits = waits[:limit], waits[limit:]
                    nop = mybir.InstNoOp(name=f"{inst.name}-ws{k}", ins=[], outs=[])
                    nop.engine = inst.engine
                    nop.sync_info = mybir.SyncInfo(on_wait=chunk, on_update=[])
                    nc.register_instruction(nop)
                    new.append(nop)
                    k += 1
                if k:
                    si.on_wait = waits
                new.append(inst)
                ups = list(si.on_update) if (si is not None and si.on_update) else []
                if len(ups) > limit and type(inst).__name__ not in (
                    "InstDMA", "InstDMACopy", "InstTensorLoad", "InstTensorSave",
                ):
                    si.on_update = ups[:limit]
                    for j, up in enumerate(ups[limit:]):
                        nop = mybir.InstNoOp(name=f"{inst.name}-us{j}", ins=[], outs=[])
                        nop.engine = inst.engine
                        nop.sync_info = mybir.SyncInfo(on_wait=[], on_update=[up])
                        nc.register_instruction(nop)
                        new.append(nop)
            bb.instructions[:] = new


def build_nc(s=S, repeat=1):
    st = s // P             # number of 128-wide k tiles
    qc_w = min(512, s)      # q chunk width
    nqc = s // qc_w

    nc = bass.Bass()
    xb = nc.dram_tensor("xb", [P, HT, s], BF16, kind="ExternalInput")
    wqb = nc.dram_tensor("wqb", [P, HT, EW], BF16, kind="ExternalInput")
    wkub = nc.dram_tensor("wkub", [P, HT, HD + GQ], BF16, kind="ExternalInput")
    mkf = nc.dram_tensor("mkf", [P, st], F32, kind="ExternalInput")
    uscr = [nc.dram_tensor(f"uscr{i}", [GQ, s], BF16) for i in range(2)]
    out = nc.dram_tensor("out", [GQ, 2, s], F32, kind="ExternalOutput")

    with tile.TileContext(nc) as tc:
        with (
            tc.tile_pool(name="persist", bufs=1) as persist,
            tc.tile_pool(name="ep", bufs=8) as ep,
            tc.tile_pool(name="obp", bufs=2) as obp,
            tc.tile_pool(name="psum_s", bufs=3, space="PSUM") as psum_s,
            tc.tile_pool(name="psum_p", bufs=1, space="PSUM") as psum_p,
            tc.tile_pool(name="psum_o", bufs=1, space="PSUM") as psum_o,
        ):
            xb_sb = persist.tile([P, HT, s], BF16)
            wqb_sb = persist.tile([P, HT, EW], BF16)
            wkub_sb = persist.tile([P, HT, HD + GQ], BF16)
            mkf_sb = persist.tile([P, st], F32)
            # per-iteration-parity projection outputs
            KT2 = [persist.tile([P, s], BF16, tag=f"kt2_{i}", name=f"kt2_{i}")
                   for i in range(2)]
            QT = [[persist.tile([P, s], BF16, tag=f"qt_{i}_{p}", name=f"qt_{i}_{p}")
                   for p in range(2)] for i in range(2)]
            KTU = [persist.tile([P, s], BF16, tag=f"ktu_{i}", name=f"ktu_{i}")
                   for i in range(2)]
            MUB = [persist.tile([P, st, 2 * GQ], BF16, tag=f"mub_{i}",
                                name=f"mub_{i}") for i in range(2)]
            UST = [persist.tile([P, st, GQ], BF16, tag=f"ust_{i}",
                                name=f"ust_{i}") for i in range(2)]

            nc.sync.dma_start(out=wqb_sb[:], in_=wqb[:, :, :])
            nc.sync.dma_start(out=wkub_sb[:], in_=wkub[:, :, :])
            nc.sync.dma_start(out=mkf_sb[:], in_=mkf[:, :])
            for q in range(nqc):
                sl = slice(q * qc_w, (q + 1) * qc_w)
                nc.sync.dma_start(out=xb_sb[:, :, sl], in_=xb[:, :, sl])

            def proj_stream(par):
                """Thunk stream for iteration-parity `par`'s projections.
                Each thunk emits at most one PE matmul so the stream can be
                drip-fed between score tiles without stalling the exp
                engines.  Group order: 4 K+U chunks, 8 Q chunk-halves, then
                the MUB u-scatter."""
                for gi in range(4):
                    sl = slice(gi * qc_w, (gi + 1) * qc_w)
                    box = [None]

                    def mm(t, sl=sl, box=box):
                        if t == 0:
                            box[0] = psum_p.tile([P, qc_w], F32, tag="psp", name="psp")
                        nc.tensor.matmul(
                            box[0][0:HD + GQ, :], lhsT=wkub_sb[:, t, :],
                            rhs=xb_sb[:, t, sl],
                            start=(t == 0), stop=(t == HT - 1))

                    for t in range(HT):
                        yield (lambda t=t, mm=mm: mm(t))

                    def fin(sl=sl, box=box):
                        # single merged cast (k rows 0-63 + u rows 64-67);
                        # the fan-out below is DMA-only
                        nc.scalar.copy(KTU[par][0:HD + GQ, sl],
                                       box[0][0:HD + GQ, :])
                        nc.sync.dma_start(out=KT2[par][0:HD, sl],
                                          in_=KTU[par][0:HD, sl])
                        nc.sync.dma_start(out=KT2[par][HD:P, sl],
                                          in_=KTU[par][0:HD, sl])
                        nc.sync.dma_start(out=uscr[par][:, sl],
                                          in_=KTU[par][HD:HD + GQ, sl])

                    yield fin
                for q8 in range(nqc):
                    for p2 in range(2):
                        sl = slice(q8 * qc_w, (q8 + 1) * qc_w)
                        box = [None]

                        def mm(t, sl=sl, p2=p2, box=box):
                            if t == 0:
                                box[0] = psum_p.tile([P, qc_w], F32, tag="psp", name="psp")
                            nc.tensor.matmul(
                                box[0][:, :],
                                lhsT=wqb_sb[:, t, p2 * P:(p2 + 1) * P],
                                rhs=xb_sb[:, t, sl],
                                start=(t == 0), stop=(t == HT - 1))

                        for t in range(HT):
                            yield (lambda t=t, mm=mm: mm(t))

                        def fin(sl=sl, p2=p2, box=box):
                            nc.scalar.copy(QT[par][p2][:, sl], box[0][:, :])

                        yield fin

                def mub():
                    for j in range(GQ):
                        nc.sync.dma_start(
                            out=UST[par][:, :, j],
                            in_=uscr[par][j, :].rearrange("(t p) -> p t", p=P))
                        nc.gpsimd.tensor_tensor(
                            out=MUB[par][:, :, 2 * j], in0=UST[par][:, :, j],
                            in1=mkf_sb[:, :], op=OP.mult)
                        nc.gpsimd.tensor_copy(MUB[par][:, :, 2 * j + 1],
                                              mkf_sb[:, :])

                yield mub

            def emit_av(po, mub_par, pk, pe1, pe2, start, stop):
                for h, (pe, c) in enumerate(
                    ((pe1, 0), (pe1, 1), (pe2, 0), (pe2, 1))
                ):
                    nc.tensor.matmul(
                        po[32 * h:32 * h + 2, :],
                        lhsT=MUB[mub_par][:, pk, 2 * h:2 * h + 2],
                        rhs=pe[:, c, :],
                        start=start, stop=stop,
                        tile_position=(0, 32 * h))

            # iteration 0's projections run up front (lead-in, paid once)
            for th in proj_stream(0):
                th()

            # carry: finished chunk awaiting its last AV flush + drain,
            # as (po, qsl, mub_par, pk, pe1, pe2)
            carry = None
            stream = iter(())

            for it in range(repeat):
                cur = it % 2
                nxt = (it + 1) % 2
                if it + 1 < repeat:
                    stream = iter(list(proj_stream(nxt)))
                for q in range(nqc):
                    qsl = slice(q * qc_w, (q + 1) * qc_w)
                    po = psum_o.tile([P, qc_w], F32, tag="po")
                    if carry is None:
                        # very first chunk: nothing to drain, memset directly
                        nc.vector.memset(po[:, :], 0.0)
                    prev = None
                    for kt in range(st):
                        ksl = slice(kt * P, (kt + 1) * P)
                        es = []
                        for hp in range(2):
                            ps = psum_s.tile([P, 2, qc_w], F32, tag="ps")
                            nc.tensor.matmul(
                                ps[:, 0, :], lhsT=KT2[cur][0:HD, ksl],
                                rhs=QT[cur][hp][0:HD, qsl], start=True,
                                stop=True, tile_position=(0, 0))
                            nc.tensor.matmul(
                                ps[:, 1, :], lhsT=KT2[cur][HD:P, ksl],
                                rhs=QT[cur][hp][HD:P, qsl], start=True,
                                stop=True, tile_position=(HD, 0))
                            e = ep.tile([P, 2, qc_w], BF16, tag="e")
                            if hp == 0:
                                nc.scalar.activation(
                                    e[:, :, :], ps[:, :, :], AF.Exp,
                                    scale=1.0 / 8.0)
                            else:
                                nc.vector.tensor_scalar(
                                    out=e[:, :, :].bitcast(U16),
                                    in0=ps[:, :, :], scalar1=SCH_A,
                                    scalar2=SCH_B, op0=OP.mult, op1=OP.add)
                            es.append(e)
                        if kt == 1 and carry is not None:
                            # flush the previous chunk's last AV, drain its
                            # accumulator, and (same 1-bank slot) memset ours.
                            # Done after this chunk's first scores so neither
                            # exp engine goes idle over the boundary.
                            cpo, cqsl, cpar, cpk, cp1, cp2 = carry
                            emit_av(cpo, cpar, cpk, cp1, cp2,
                                    start=(cpk == 0), stop=True)
                            ob = obp.tile([P, qc_w], F32, tag="ob")
                            nc.vector.tensor_copy(ob[:, :], cpo[:, :])
                            nc.vector.memset(po[:, :], 0.0)
                            for h in range(GQ):
                                nc.sync.dma_start(
                                    out=out[h, :, cqsl],
                                    in_=ob[32 * h:32 * h + 2, :])
                            carry = None
                        if prev is not None:
                            pk, pe1, pe2 = prev
                            emit_av(po, cur, pk, pe1, pe2,
                                    start=(pk == 0), stop=False)
                        prev = (kt, es[0], es[1])
                        if kt >= 2:
                            for _ in range(2):
                                th = next(stream, None)
                                if th is not None:
                                    th()
                    pk, pe1, pe2 = prev
                    carry = (po, qsl, cur, pk, pe1, pe2)

            # final chunk: flush + drain (no successor)
            cpo, cqsl, cpar, cpk, cp1, cp2 = carry
            emit_av(cpo, cpar, cpk, cp1, cp2, start=(cpk == 0), stop=True)
            ob = obp.tile([P, qc_w], F32, tag="ob")
            nc.vector.tensor_copy(ob[:, :], cpo[:, :])
            for h in range(GQ):
                nc.sync.dma_start(
                    out=out[h, :, cqsl], in_=ob[32 * h:32 * h + 2, :])

    _split_excess_waits(nc)
    return nc


_NC_CACHE = {}


def _get_nc(s=S, repeat=1):
    key = (s, repeat)
    if key not in _NC_CACHE:
        _NC_CACHE[key] = build_nc(s, repeat)
    return _NC_CACHE[key]


def make_inputs(hidden_states, attention_mask, wq, wk, wv, wo, s=S):
    """Host-side shard prep: per-core input dicts (bf16, h-tiled)."""
    hidden_states = np.asarray(hidden_states, dtype=np.float32)
    attention_mask = np.asarray(attention_mask)
    wq = np.asarray(wq, dtype=np.float32)
    wk = np.asarray(wk, dtype=np.float32)
    wv = np.asarray(wv, dtype=np.float32)
    wo = np.asarray(wo, dtype=np.float32)
    st = s // P

    def h_pack(a):
        # [H, C] -> [128, HT, C]  with h = 128*t + j
        c = a.shape[1]
        return np.ascontiguousarray(a.reshape(HT, P, c).transpose(1, 0, 2))

    in_maps = []
    for core in range(NCORES):
        b, g = divmod(core, NKV)
        xT = np.ascontiguousarray(hidden_states[b, :s, :].T)      # [H, s]
        xb = h_pack(xT).astype(NPBF)
        wq_g = wq[:, g * EW:(g + 1) * EW]
        wqb = h_pack(wq_g).astype(NPBF)
        wk_g = wk[:, g * HD:(g + 1) * HD]
        wo_g = wo[g * EW:(g + 1) * EW, 0].reshape(GQ, HD).T        # [HD, GQ]
        wu_g = wv[:, g * HD:(g + 1) * HD] @ wo_g
        wkub = h_pack(np.concatenate([wk_g, wu_g], axis=1)).astype(NPBF)
        m = (attention_mask[b, :s] != 0).astype(np.float32)
        mkf = np.ascontiguousarray(m.reshape(st, P).T)             # [128, st]
        in_maps.append({"xb": xb, "wqb": wqb, "wkub": wkub, "mkf": mkf})
    return in_maps


def combine(results, s=S):
    """Host-side gather: out[b,q] = sum over group cores and heads num/den."""
    out = np.zeros((B, s, 1), dtype=np.float32)
    for core in range(NCORES):
        b = core // NKV
        nd = results[core]["out"]          # [GQ, 2, s]
        out[b, :, 0] += (nd[:, 0, :] / nd[:, 1, :]).sum(axis=0)
    return out


def kernel(hidden_states, attention_mask, wq, wk, wv, wo):
    nc = _get_nc()
    in_maps = make_inputs(hidden_states, attention_mask, wq, wk, wv, wo)
    res = run_bass_kernel_spmd(nc, in_maps, core_ids=list(range(NCORES)))
    return combine(res.results)


# revision 11
# speedup vs baseline: 1.4947x; 1.0016x over previous
"""Trainium2 Bass kernel for CoEncoderDynamicAttention (v5: col-tiled AV,
cross-iteration software pipelining, balanced ACT/DVE exp).

Model (reference):
  q = x @ wq   -> [B,S,NH,HD];  k = x @ wk -> [B,S,NKV,HD];  v = x @ wv
  scores = q k^T / sqrt(HD), masked, softmax over k
  out = (attn @ v) reshaped @ wo        (wo: [NH*HD, 1])

Sharding: 8 cores = (batch b in 0..1) x (kv-group g in 0..3).  Each kv
group owns 1 kv head and GQ=4 q heads.  Since wo has output dim 1, fold
wo into v on the host:  u_h = v_g @ wo_h, so per-core output is
  num_h[q] = sum_k m[k] u_h[k] e_h[k,q],  den_h[q] = sum_k m[k] e_h[k,q]
with e = exp(s/sqrt(HD)); the mask is folded multiplicatively into the
AV stationary operand.  Host combines out[b,q] = sum_{g,h} num_h/den_h.

Perf structure (the kernel is exp-bound: every scores element must leave
PSUM through ACT or DVE, ~277G elem/s total):
  * Scores matmuls pair heads in disjoint PE row groups (KT duplicated
    to partitions 64-127); both pairs (heads 0,1 / 2,3) per k-tile.
  * AV matmuls (M=2: num/den) are 4x COLUMN-TILED: head h ->
    tile_position (0, 32h), output partitions 32h..32h+1 of ONE 1-bank
    PSUM accumulator; all four stream concurrently.
  * exp strictly alternates ACT (native Exp) / DVE (Schraudolph-to-bf16
    bit trick) so both engines stream continuously.
  * Cross-iteration pipelining: KT2/QT/MUB/uscr are double-buffered by
    iteration parity and the NEXT iteration's K/Q projection matmul
    groups are emitted interleaved (one per 4 k-tiles) inside the
    current iteration's attention, filling the PE's idle slots, so the
    exp engines never wait for projections at iteration boundaries.
  * The per-q-chunk output copy is emitted two k-tiles INTO the next
    chunk so it never blocks the DVE exp stream at chunk boundaries.
  * AV for k-tile t is emitted after the scores matmuls of k-tile t+1
    so the tensor queue never waits on the exp engines.

PSUM budget (8 banks): 3 x 2-bank score tiles + 1-bank projection tile
+ 1-bank AV accumulator = 8.
"""

import numpy as np
import ml_dtypes

import concourse.bass as bass
import concourse.mybir as mybir
import concourse.tile as tile
from concourse.bass_utils import run_bass_kernel_spmd

B, S, H = 2, 2048, 1024
NH, NKV, HD = 16, 4, 64
GQ = NH // NKV          # q heads per kv group
EW = GQ * HD            # per-core q projection width (256)
NCORES = 8
P = 128
HT = H // P             # h (contraction) tiles
F32 = mybir.dt.float32
BF16 = mybir.dt.bfloat16
U16 = mybir.dt.uint16
AF = mybir.ActivationFunctionType
OP = mybir.AluOpType
NPBF = ml_dtypes.bfloat16

# Schraudolph-to-bf16: for psum score ps (= 8*s_true),
# u16 = round(ps*SCH_A + SCH_B) is the bf16 bit pattern of ~exp(s_true).
SCH_A = float(128.0 * np.log2(np.e) / 8.0)
SCH_B = float(16256.0 - 7.373)


def _split_excess_waits(nc, limit=1):
    """This walrus build only accepts one sync-wait (and update) per
    instruction; hoist extras onto NoOps on the same engine."""
    for f in nc.m.functions:
        for bb in f.blocks:
            new = []
            for inst in bb.instructions:
                si = getattr(inst, "sync_info", None)
                waits = list(si.on_wait) if (si is not None and si.on_wait) else []
                k = 0
                while len(waits) > limit:
                    chunk, waits = waits[:limit], waits[limit:]
                    nop = mybir.InstNoOp(name=f"{inst.name}-ws{k}", ins=[], outs=[])
                    nop.engine = inst.engine
                    nop.sync_info = mybir.SyncInfo(on_wait=chunk, on_update=[])
                    nc.register_instruction(nop)
                    new.append(nop)
                    k += 1
                if k:
                    si.on_wait = waits
                new.append(inst)
                ups = list(si.on_update) if (si is not None and si.on_update) else []
                if len(ups) > limit and type(inst).__name__ not in (
                    "InstDMA", "InstDMACopy", "InstTensorLoad", "InstTensorSave",
                ):
                    si.on_update = ups[:limit]
                    for j, up in enumerate(ups[limit:]):
                        nop = mybir.InstNoOp(name=f"{inst.name}-us{j}", ins=[], outs=[])
                        nop.engine = inst.engine
                        nop.sync_info = mybir.SyncInfo(on_wait=[], on_update=[up])
                        nc.register_instruction(nop)
                        new.append(nop)
            bb.instructions[:] = new


def build_nc(s=S, repeat=1, sim_safe=False):
    st = s // P             # number of 128-wide k tiles
    qc_w = min(512, s)      # q chunk width
    nqc = s // qc_w

    nc = bass.Bass()
    xb = nc.dram_tensor("xb", [P, HT, s], BF16, kind="ExternalInput")
    wqb = nc.dram_tensor("wqb", [P, HT, EW], BF16, kind="ExternalInput")
    wkub = nc.dram_tensor("wkub", [P, HT, HD + GQ], BF16, kind="ExternalInput")
    mkf = nc.dram_tensor("mkf", [P, st], F32, kind="ExternalInput")
    uscr = [nc.dram_tensor(f"uscr{i}", [GQ, s], BF16) for i in range(2)]
    out = nc.dram_tensor("out", [GQ, 2, s], F32, kind="ExternalOutput")

    with tile.TileContext(nc) as tc:
        with (
            tc.tile_pool(name="persist", bufs=1) as persist,
            tc.tile_pool(name="ep", bufs=8) as ep,
            tc.tile_pool(name="obp", bufs=2) as obp,
            tc.tile_pool(name="psum_s", bufs=3, space="PSUM") as psum_s,
            tc.tile_pool(name="psum_p", bufs=1, space="PSUM") as psum_p,
            tc.tile_pool(name="psum_o", bufs=1, space="PSUM") as psum_o,
        ):
            xb_sb = persist.tile([P, HT, s], BF16)
            wqb_sb = persist.tile([P, HT, EW], BF16)
            wkub_sb = persist.tile([P, HT, HD + GQ], BF16)
            mkf_sb = persist.tile([P, st], F32)
            # per-iteration-parity projection outputs
            KT2 = [persist.tile([P, s], BF16, tag=f"kt2_{i}", name=f"kt2_{i}")
                   for i in range(2)]
            QT = [[persist.tile([P, s], BF16, tag=f"qt_{i}_{p}", name=f"qt_{i}_{p}")
                   for p in range(2)] for i in range(2)]
            KTU = [persist.tile([P, s], BF16, tag=f"ktu_{i}", name=f"ktu_{i}")
                   for i in range(2)]
            MUB = [persist.tile([P, st, 2 * GQ], BF16, tag=f"mub_{i}",
                                name=f"mub_{i}") for i in range(2)]
            UST = [persist.tile([P, st, GQ], BF16, tag=f"ust_{i}",
                                name=f"ust_{i}") for i in range(2)]

            nc.sync.dma_start(out=wqb_sb[:], in_=wqb[:, :, :])
            nc.sync.dma_start(out=wkub_sb[:], in_=wkub[:, :, :])
            nc.sync.dma_start(out=mkf_sb[:], in_=mkf[:, :])
            for q in range(nqc):
                sl = slice(q * qc_w, (q + 1) * qc_w)
                nc.sync.dma_start(out=xb_sb[:, :, sl], in_=xb[:, :, sl])

            def proj_stream(par):
                """Thunk stream for iteration-parity `par`'s projections.
                Each thunk emits at most one PE matmul so the stream can be
                drip-fed between score tiles without stalling the exp
                engines.  Group order: 4 K+U chunks, 8 Q chunk-halves, then
                the MUB u-scatter."""
                for gi in range(4):
                    sl = slice(gi * qc_w, (gi + 1) * qc_w)
                    box = [None]

                    def mm(t, sl=sl, box=box):
                        if t == 0:
                            box[0] = psum_p.tile([P, qc_w], F32, tag="psp", name="psp")
                        nc.tensor.matmul(
                            box[0][0:HD + GQ, :], lhsT=wkub_sb[:, t, :],
                            rhs=xb_sb[:, t, sl],
                            start=(t == 0), stop=(t == HT - 1))

                    for t in range(HT):
                        yield (lambda t=t, mm=mm: mm(t))

                    def fin(sl=sl, box=box):
                        # single merged cast (k rows 0-63 + u rows 64-67);
                        # the fan-out below is DMA-only
                        nc.scalar.copy(KTU[par][0:HD + GQ, sl],
                                       box[0][0:HD + GQ, :])
                        nc.sync.dma_start(out=KT2[par][0:HD, sl],
                                          in_=KTU[par][0:HD, sl])
                        nc.sync.dma_start(out=KT2[par][HD:P, sl],
                                          in_=KTU[par][0:HD, sl])
                        nc.sync.dma_start(out=uscr[par][:, sl],
                                          in_=KTU[par][HD:HD + GQ, sl])

                    yield fin
                for q8 in range(nqc):
                    for p2 in range(2):
                        sl = slice(q8 * qc_w, (q8 + 1) * qc_w)
                        box = [None]

                        def mm(t, sl=sl, p2=p2, box=box):
                            if t == 0:
                                box[0] = psum_p.tile([P, qc_w], F32, tag="psp", name="psp")
                            nc.tensor.matmul(
                                box[0][:, :],
                                lhsT=wqb_sb[:, t, p2 * P:(p2 + 1) * P],
                                rhs=xb_sb[:, t, sl],
                                start=(t == 0), stop=(t == HT - 1))

                        for t in range(HT):
                            yield (lambda t=t, mm=mm: mm(t))

                        def fin(sl=sl, p2=p2, box=box):
                            nc.scalar.copy(QT[par][p2][:, sl], box[0][:, :])

                        yield fin

                def mub():
                    for j in range(GQ):
                        nc.sync.dma_start(
                            out=UST[par][:, :, j],
                            in_=uscr[par][j, :].rearrange("(t p) -> p t", p=P))
                        nc.gpsimd.tensor_tensor(
                            out=MUB[par][:, :, 2 * j], in0=UST[par][:, :, j],
                            in1=mkf_sb[:, :], op=OP.mult)
                        nc.gpsimd.tensor_copy(MUB[par][:, :, 2 * j + 1],
                                              mkf_sb[:, :])

                yield mub

            def emit_av(po, mub_par, pk, pe1, pe2, start, stop):
                for h, (pe, c) in enumerate(
                    ((pe1, 0), (pe1, 1), (pe2, 0), (pe2, 1))
                ):
                    nc.tensor.matmul(
                        po[32 * h:32 * h + 2, :],
                        lhsT=MUB[mub_par][:, pk, 2 * h:2 * h + 2],
                        rhs=pe[:, c, :],
                        start=start, stop=stop,
                        tile_position=(0, 32 * h))

            # iteration 0's projections run up front (lead-in, paid once)
            for th in proj_stream(0):
                th()

            # carry: finished chunk awaiting its last AV flush + drain,
            # as (po, qsl, mub_par, pk, pe1, pe2)
            carry = None
            stream = iter(())

            for it in range(repeat):
                cur = it % 2
                nxt = (it + 1) % 2
                if it + 1 < repeat:
                    stream = iter(list(proj_stream(nxt)))
                for q in range(nqc):
                    qsl = slice(q * qc_w, (q + 1) * qc_w)
                    po = psum_o.tile([P, qc_w], F32, tag="po")
                    if sim_safe and carry is None:
                        # sim-only: define the partitions the col-tiled AV
                        # matmuls skip so the full-tile drain copy reads
                        # initialized data (hw reads-and-discards garbage)
                        nc.vector.memset(po[:, :], 0.0)
                    prev = None
                    for kt in range(st):
                        ksl = slice(kt * P, (kt + 1) * P)
                        es = []
                        for hp in range(2):
                            ps = psum_s.tile([P, 2, qc_w], F32, tag="ps")
                            nc.tensor.matmul(
                                ps[:, 0, :], lhsT=KT2[cur][0:HD, ksl],
                                rhs=QT[cur][hp][0:HD, qsl], start=True,
                                stop=True, tile_position=(0, 0))
                            nc.tensor.matmul(
                                ps[:, 1, :], lhsT=KT2[cur][HD:P, ksl],
                                rhs=QT[cur][hp][HD:P, qsl], start=True,
                                stop=True, tile_position=(HD, 0))
                            e = ep.tile([P, 2, qc_w], BF16, tag="e")
                            if hp == 0:
                                nc.scalar.activation(
                                    e[:, :, :], ps[:, :, :], AF.Exp,
                                    scale=1.0 / 8.0)
                            else:
                                nc.vector.tensor_scalar(
                                    out=e[:, :, :].bitcast(U16),
                                    in0=ps[:, :, :], scalar1=SCH_A,
                                    scalar2=SCH_B, op0=OP.mult, op1=OP.add)
                            es.append(e)
                        if kt == 1 and carry is not None:
                            # flush the previous chunk's last AV, drain its
                            # accumulator, and (same 1-bank slot) memset ours.
                            # Done after this chunk's first scores so neither
                            # exp engine goes idle over the boundary.
                            cpo, cqsl, cpar, cpk, cp1, cp2 = carry
                            emit_av(cpo, cpar, cpk, cp1, cp2,
                                    start=(cpk == 0), stop=True)
                            ob = obp.tile([P, qc_w], F32, tag="ob")
                            # alternate the drain engine to balance ACT/DVE
                            if q % 2 == 0:
                                nc.vector.tensor_copy(ob[:, :], cpo[:, :])
                            else:
                                nc.scalar.copy(ob[:, :], cpo[:, :])
                            if sim_safe:
                                nc.vector.memset(po[:, :], 0.0)
                            for h in range(GQ):
                                nc.sync.dma_start(
                                    out=out[h, :, cqsl],
                                    in_=ob[32 * h:32 * h + 2, :])
                            carry = None
                        if prev is not None:
                            pk, pe1, pe2 = prev
                            emit_av(po, cur, pk, pe1, pe2,
                                    start=(pk == 0), stop=False)
                        prev = (kt, es[0], es[1])
                        if kt >= 2:
                            for _ in range(2):
                                th = next(stream, None)
                                if th is not None:
                                    th()
                    pk, pe1, pe2 = prev
                    carry = (po, qsl, cur, pk, pe1, pe2)

            # final chunk: flush + drain (no successor)
            cpo, cqsl, cpar, cpk, cp1, cp2 = carry
            emit_av(cpo, cpar, cpk, cp1, cp2, start=(cpk == 0), stop=True)
            ob = obp.tile([P, qc_w], F32, tag="ob")
            nc.vector.tensor_copy(ob[:, :], cpo[:, :])
            for h in range(GQ):
                nc.sync.dma_start(
                    out=out[h, :, cqsl], in_=ob[32 * h:32 * h + 2, :])

    _split_excess_waits(nc)
    return nc


_NC_CACHE = {}


def _get_nc(s=S, repeat=1, sim_safe=False):
    key = (s, repeat, sim_safe)
    if key not in _NC_CACHE:
        _NC_CACHE[key] = build_nc(s, repeat, sim_safe)
    return _NC_CACHE[key]


def make_inputs(hidden_states, attention_mask, wq, wk, wv, wo, s=S):
    """Host-side shard prep: per-core input dicts (bf16, h-tiled)."""
    hidden_states = np.asarray(hidden_states, dtype=np.float32)
    attention_mask = np.asarray(attention_mask)
    wq = np.asarray(wq, dtype=np.float32)
    wk = np.asarray(wk, dtype=np.float32)
    wv = np.asarray(wv, dtype=np.float32)
    wo = np.asarray(wo, dtype=np.float32)
    st = s // P

    def h_pack(a):
        # [H, C] -> [128, HT, C]  with h = 128*t + j
        c = a.shape[1]
        return np.ascontiguousarray(a.reshape(HT, P, c).transpose(1, 0, 2))

    in_maps = []
    for core in range(NCORES):
        b, g = divmod(core, NKV)
        xT = np.ascontiguousarray(hidden_states[b, :s, :].T)      # [H, s]
        xb = h_pack(xT).astype(NPBF)
        wq_g = wq[:, g * EW:(g + 1) * EW]
        wqb = h_pack(wq_g).astype(NPBF)
        wk_g = wk[:, g * HD:(g + 1) * HD]
        wo_g = wo[g * EW:(g + 1) * EW, 0].reshape(GQ, HD).T        # [HD, GQ]
        wu_g = wv[:, g * HD:(g + 1) * HD] @ wo_g
        wkub = h_pack(np.concatenate([wk_g, wu_g], axis=1)).astype(NPBF)
        m = (attention_mask[b, :s] != 0).astype(np.float32)
        mkf = np.ascontiguousarray(m.reshape(st, P).T)             # [128, st]
        in_maps.append({"xb": xb, "wqb": wqb, "wkub": wkub, "mkf": mkf})
    return in_maps


def combine(results, s=S):
    """Host-side gather: out[b,q] = sum over group cores and heads num/den."""
    out = np.zeros((B, s, 1), dtype=np.float32)
    for core in range(NCORES):
        b = core // NKV
        nd = results[core]["out"]          # [GQ, 2, s]
        out[b, :, 0] += (nd[:, 0, :] / nd[:, 1, :]).sum(axis=0)
    return out


def kernel(hidden_states, attention_mask, wq, wk, wv, wo):
    nc = _get_nc()
    in_maps = make_inputs(hidden_states, attention_mask, wq, wk, wv, wo)
    res = run_bass_kernel_spmd(nc, in_maps, core_ids=list(range(NCORES)))
    return combine(res.results)
